# revision 6
# baseline (speedup 1.0000x reference)
"""Trainium2 Bass kernel for nn_MultiHeadAttention_26225070309648 (sparse_attention).

Full (unsharded) inputs in, full output out. Data-parallel over batch: each of
the 8 NeuronCores processes one batch element (B == 8).

Key algebraic transform: the reference applies a 31-tap conv along the QUERY
axis of the scores (then softmax). Conv over q commutes with the k-contraction,
so conv(scores) == (A @ Q) @ K.T where A is the banded [T, T] window matrix
(zero-padded at the edges, exactly matching conv2d zero padding). Folding the
1/sqrt(dh) scale into A as well:  S_final = (A/8 @ (x Wq + 1 bq)) @ K.T
                                          = (x~ Wq + A1 bq) @ K.T
with x~ = (A/8) @ x computed once per core (banded matmul, cheap) and
A1 = (A/8) @ 1 (row sums). Scores are tiny (|S| < 5), so softmax needs no
max-subtraction; the denominator comes for free as a 65th "ones" column of V.
"""
import numpy as np

T = 1024
DIM = 512
H = 8
DH = 64
WIN = 31
FRAC = 0.3
B = 8
NT = T // 128   # 8 token tiles
NC = DIM // 128  # 4 channel tiles
F32 = np.float32


def _band_consts():
    """ATb[j, s] = (A/8)[(j-1+s)*128 : .., j*128 : ..] block (zeros out of range);
    A1 = (A/8) @ ones (row sums of the zero-padded band)."""
    n = (WIN - 1) // 2
    side = np.array([(1.0 - FRAC) ** i for i in range(n, 0, -1)], dtype=F32)
    w = np.concatenate([side, np.array([1.0], dtype=F32), side[::-1]])
    A = np.zeros((T, T), dtype=F32)
    for j in range(WIN):
        off = j - n
        A += np.diag(np.full(T - abs(off), w[j], dtype=F32), k=off)
    A /= np.sqrt(F32(DH))
    A1 = A.sum(axis=1).astype(F32)
    ATb = np.zeros((NT, 3, 128, 128), dtype=F32)
    for jt in range(NT):
        for s in range(3):
            rt = jt - 1 + s
            if 0 <= rt < NT:
                # rhs[q, q'] for out-tile jt, K-tile rt: A[q, q'] (A symmetric)
                ATb[jt, s] = A[rt * 128:(rt + 1) * 128, jt * 128:(jt + 1) * 128]
    return ATb, A1


def _emit(tc, ins, outs):
    import concourse.bass as bass
    import concourse.mybir as mybir
    from concourse.masks import make_identity

    nc = tc.nc
    dt = mybir.dt.float32
    x_d, wq_d, wk_d, wv_d, wo_d, bq_d, bk_d, bv_d, bo_d, atb_d, a1_d = ins
    y_d = outs

    from contextlib import ExitStack
    ctx = ExitStack()
    cn = ctx.enter_context(tc.tile_pool(name="cn", bufs=1))

    # ---- constants / inputs to SBUF ----
    x_sb = cn.tile([128, NT, DIM], dt)          # x[j*128+p, c]
    nc.sync.dma_start(out=x_sb, in_=x_d.rearrange("(j p) c -> p j c", p=128))
    atb_sb = cn.tile([128, NT, 3, 128], dt)
    nc.sync.dma_start(out=atb_sb, in_=atb_d.rearrange("j s p c -> p j s c"))
    wq_sb = cn.tile([128, NC, DIM], dt)
    nc.sync.dma_start(out=wq_sb, in_=wq_d.rearrange("(a p) i -> p a i", p=128))
    wk_sb = cn.tile([128, NC, DIM], dt)
    nc.sync.dma_start(out=wk_sb, in_=wk_d.rearrange("(a p) i -> p a i", p=128))
    wv_sb = cn.tile([128, NC, DIM], dt)
    nc.sync.dma_start(out=wv_sb, in_=wv_d.rearrange("(a p) i -> p a i", p=128))
    wo_sb = cn.tile([128, NC, DIM], dt)
    nc.sync.dma_start(out=wo_sb, in_=wo_d.rearrange("(a p) i -> p a i", p=128))
    a1_sb = cn.tile([1, T], dt)
    nc.sync.dma_start(out=a1_sb, in_=a1_d)
    bq_sb = cn.tile([1, DIM], dt)
    nc.sync.dma_start(out=bq_sb, in_=bq_d)
    bv_sb = cn.tile([1, DIM], dt)
    nc.sync.dma_start(out=bv_sb, in_=bv_d)
    bo_sb = cn.tile([1, DIM], dt)
    nc.sync.dma_start(out=bo_sb, in_=bo_d)
    bk_sb = cn.tile([128, NC], dt)              # bk[a*128+p] -> [p, a]
    nc.sync.dma_start(out=bk_sb, in_=bk_d.rearrange("(a p) -> p a", p=128))

    ident = cn.tile([128, 128], dt)
    make_identity(nc, ident[:])
    ones_sb = cn.tile([1, 128], dt)
    nc.vector.memset(ones_sb, 1.0)

    # ---- big persistent intermediates ----
    xT_sb = cn.tile([128, NC, T], dt)    # x.T   [c, t]
    xtT_sb = cn.tile([128, NC, T], dt)   # x~.T  [c, q']
    qtT_sb = cn.tile([128, NC, T], dt)   # Q~.T  [i, q']
    kT_sb = cn.tile([128, NC, T], dt)    # K.T   [i, k]
    VA = 65
    v_sb = cn.tile([128, NT, H * VA], dt)  # V augmented: per head 64 vals + ones col
    ov = v_sb.rearrange("p j (h u) -> p j h u", u=VA)
    nc.vector.memset(ov[:, :, :, 64:65], 1.0)
    oT_sb = cn.tile([128, NC, T], dt)    # normalized attention out, T-layout [i, t]

    with tc.tile_pool(name="ps_a", bufs=4, space="PSUM") as ps_a:
        # ---- phase 1: xT = x.T via PE identity matmuls ----
        for ct in range(NC):
            for jg in range(2):  # groups of 4 token tiles -> one [128, 512] psum
                p = ps_a.tile([128, 512], dt)
                for q in range(4):
                    jt = jg * 4 + q
                    nc.tensor.matmul(p[:, q * 128:(q + 1) * 128],
                                     x_sb[:, jt, ct * 128:(ct + 1) * 128],
                                     ident[:], start=True, stop=True)
                nc.vector.tensor_copy(xT_sb[:, ct, jg * 512:(jg + 1) * 512], p[:])

        # ---- phase 2: x~T[c, q'] = sum_q x[q, c] * (A/8)[q, q'] (banded) ----
        for ct in range(NC):
            for jg in range(2):
                p = ps_a.tile([128, 512], dt)
                for q in range(4):
                    jt = jg * 4 + q
                    svalid = [s for s in range(3) if 0 <= jt - 1 + s < NT]
                    for si, s in enumerate(svalid):
                        nc.tensor.matmul(
                            p[:, q * 128:(q + 1) * 128],
                            x_sb[:, jt - 1 + s, ct * 128:(ct + 1) * 128],
                            atb_sb[:, jt, s, :],
                            start=(si == 0), stop=(si == len(svalid) - 1))
                nc.vector.tensor_copy(xtT_sb[:, ct, jg * 512:(jg + 1) * 512], p[:])

        # ---- phase 3: Q~T[i, q'] = Wq.T @ x~T + bq x A1 ----
        for it in range(NC):
            for qc in range(2):
                p = ps_a.tile([128, 512], dt)
                for ct in range(NC):
                    nc.tensor.matmul(p[:], wq_sb[:, ct, it * 128:(it + 1) * 128],
                                     xtT_sb[:, ct, qc * 512:(qc + 1) * 512],
                                     start=(ct == 0), stop=False)
                nc.tensor.matmul(p[:], bq_sb[:, it * 128:(it + 1) * 128],
                                 a1_sb[:, qc * 512:(qc + 1) * 512],
                                 start=False, stop=True)
                nc.vector.tensor_copy(qtT_sb[:, it, qc * 512:(qc + 1) * 512], p[:])

        # ---- phase 4: KT[i, k] = Wk.T @ xT  (+bk per-partition on copy-out) ----
        for it in range(NC):
            for qc in range(2):
                p = ps_a.tile([128, 512], dt)
                for ct in range(NC):
                    nc.tensor.matmul(p[:], wk_sb[:, ct, it * 128:(it + 1) * 128],
                                     xT_sb[:, ct, qc * 512:(qc + 1) * 512],
                                     start=(ct == 0), stop=(ct == NC - 1))
                nc.vector.tensor_scalar(
                    out=kT_sb[:, it, qc * 512:(qc + 1) * 512], in0=p[:],
                    scalar1=bk_sb[:, it:it + 1], scalar2=None,
                    op0=mybir.AluOpType.add)

        # ---- phase 5: V[t, i] = xT.T @ Wv + 1 x bv ----
        for jt in range(NT):
            p = ps_a.tile([128, 512], dt)
            for ct in range(NC):
                nc.tensor.matmul(p[:], xT_sb[:, ct, jt * 128:(jt + 1) * 128],
                                 wv_sb[:, ct, :], start=(ct == 0), stop=False)
            nc.tensor.matmul(p[:], ones_sb[:], bv_sb[:], start=False, stop=True)
            nc.vector.tensor_copy(
                ov[:, jt, :, 0:64],
                p[:].rearrange("p (h u) -> p h u", u=64))

    # ---- phase 6: per-head attention, heads in pairs (row-packed on PE) ----
    with tc.tile_pool(name="ps_st", bufs=2, space="PSUM") as ps_st, \
         tc.tile_pool(name="ps_av", bufs=2, space="PSUM") as ps_av, \
         tc.tile_pool(name="pexp", bufs=3) as pexp, \
         tc.tile_pool(name="dn", bufs=2) as dn:
        for g in range(H // 2):
            avs = {}
            for h in (2 * g, 2 * g + 1):
                av_t = ps_av.tile([128, T], mybir.dt.float32, tag="av", name=f"av{h}")
                avs[h] = av_t
            for kt in range(NT):
                for h in (2 * g, 2 * g + 1):
                    hb = (h % 2) * 64
                    it = h // 2
                    st = ps_st.tile([128, T], mybir.dt.float32, tag="st")
                    for qc in range(2):
                        nc.tensor.matmul(
                            st[:, qc * 512:(qc + 1) * 512],
                            kT_sb[hb:hb + 64, it, kt * 128:(kt + 1) * 128],
                            qtT_sb[hb:hb + 64, it, qc * 512:(qc + 1) * 512],
                            start=True, stop=True)
                    pe = pexp.tile([128, T], mybir.dt.float32)
                    nc.scalar.activation(out=pe[:], in_=st[:],
                                         func=mybir.ActivationFunctionType.Exp)
                    for qc in range(2):
                        nc.tensor.matmul(
                            avs[h][0:VA, qc * 512:(qc + 1) * 512],
                            v_sb[:, kt, h * VA:(h + 1) * VA],
                            pe[:, qc * 512:(qc + 1) * 512],
                            start=(kt == 0), stop=(kt == NT - 1))
            for h in (2 * g, 2 * g + 1):
                hb = (h % 2) * 64
                it = h // 2
                den = dn.tile([1, T], mybir.dt.float32, tag="den")
                nc.vector.tensor_copy(den[:], avs[h][64:65, :])
                rcp = dn.tile([1, T], mybir.dt.float32, tag="rcp")
                nc.vector.reciprocal_approx_fast(out=rcp[:], in_=den[:])
                bc = dn.tile([64, T], mybir.dt.float32, tag="bc")
                nc.gpsimd.partition_broadcast(bc[:], rcp[:])
                nc.vector.tensor_mul(oT_sb[hb:hb + 64, it, :], avs[h][0:64, :],
                                     bc[:])

    # ---- phase 7: y = oT.T @ Wo + 1 x bo ----
    with tc.tile_pool(name="ps_y", bufs=4, space="PSUM") as ps_y, \
         tc.tile_pool(name="ysb", bufs=4) as ysb:
        for jt in range(NT):
            p = ps_y.tile([128, 512], mybir.dt.float32)
            for it in range(NC):
                nc.tensor.matmul(p[:], oT_sb[:, it, jt * 128:(jt + 1) * 128],
                                 wo_sb[:, it, :], start=(it == 0), stop=False)
            nc.tensor.matmul(p[:], ones_sb[:], bo_sb[:], start=False, stop=True)
            yt = ysb.tile([128, 512], mybir.dt.float32)
            nc.vector.tensor_copy(yt[:], p[:])
            nc.sync.dma_start(out=y_d[jt * 128:(jt + 1) * 128, :], in_=yt[:])

    ctx.close()


_CACHE = {}


def _get_program():
    if "nc" in _CACHE:
        return _CACHE["nc"]
    import concourse.mybir as mybir
    import concourse.tile as tile
    from concourse import bacc

    nc = bacc.Bacc("TRN2", target_bir_lowering=False, debug=False)
    dt = mybir.dt.float32

    def din(name, shape):
        return nc.dram_tensor(name, shape, dt, kind="ExternalInput").ap()

    ins = (
        din("x", [T, DIM]),
        din("Wq", [DIM, DIM]), din("Wk", [DIM, DIM]), din("Wv", [DIM, DIM]),
        din("Wo", [DIM, DIM]),
        din("bq", [1, DIM]), din("bk", [DIM]), din("bv", [1, DIM]),
        din("bo", [1, DIM]),
        din("ATb", [NT, 3, 128, 128]), din("A1", [1, T]),
    )
    y = nc.dram_tensor("y", [T, DIM], dt, kind="ExternalOutput").ap()

    with tile.TileContext(nc) as tc:
        _emit(tc, ins, y)
    nc.compile()
    _CACHE["nc"] = nc
    return nc


def _prep_in_maps(inputs):
    ATb, A1 = _band_consts()
    x = np.ascontiguousarray(inputs["x"], dtype=F32)
    shared = {
        "Wq": np.ascontiguousarray(inputs["Wq"], F32),
        "Wk": np.ascontiguousarray(inputs["Wk"], F32),
        "Wv": np.ascontiguousarray(inputs["Wv"], F32),
        "Wo": np.ascontiguousarray(inputs["Wo"], F32),
        "bq": np.ascontiguousarray(inputs["bq"], F32).reshape(1, DIM),
        "bk": np.ascontiguousarray(inputs["bk"], F32),
        "bv": np.ascontiguousarray(inputs["bv"], F32).reshape(1, DIM),
        "bo": np.ascontiguousarray(inputs["bo"], F32).reshape(1, DIM),
        "ATb": ATb, "A1": A1.reshape(1, T),
    }
    return [{"x": x[b], **shared} for b in range(B)]


def kernel(**inputs):
    from concourse import bass_utils

    nc = _get_program()
    in_maps = _prep_in_maps(inputs)
    res = bass_utils.run_bass_kernel_spmd(nc, in_maps, core_ids=list(range(B)))
    out = np.stack([res.results[b]["y"] for b in range(B)], axis=0)
    return out.astype(F32)


# revision 7
# speedup vs baseline: 94.6454x; 94.6454x over previous
"""Trainium2 Bass kernel for nn_MultiHeadAttention_26225070309648 (sparse_attention).

Full (unsharded) inputs in, full output out. Data-parallel over batch: each of
the 8 NeuronCores processes one batch element (B == 8).

Key algebraic transform: the reference applies a 31-tap conv along the QUERY
axis of the scores (then softmax). Conv over q commutes with the k-contraction,
so conv(scores) == (A @ Q) @ K.T where A is the banded [T, T] window matrix
(zero-padded at the edges, exactly matching conv2d zero padding). Folding the
1/sqrt(dh) scale into A as well:  S_final = (A/8 @ (x Wq + 1 bq)) @ K.T
                                          = (x~ Wq + A1 bq) @ K.T
with x~ = (A/8) @ x computed once per core (banded matmul, cheap) and
A1 = (A/8) @ 1 (row sums). Scores are tiny (|S| < 5), so softmax needs no
max-subtraction; the denominator comes for free as a 65th "ones" column of V.
"""
import numpy as np

T = 1024
DIM = 512
H = 8
DH = 64
WIN = 31
FRAC = 0.3
B = 8
NT = T // 128   # 8 token tiles
NC = DIM // 128  # 4 channel tiles
F32 = np.float32


def _band_consts():
    """ATb[j, s] = (A/8)[(j-1+s)*128 : .., j*128 : ..] block (zeros out of range);
    A1 = (A/8) @ ones (row sums of the zero-padded band)."""
    n = (WIN - 1) // 2
    side = np.array([(1.0 - FRAC) ** i for i in range(n, 0, -1)], dtype=F32)
    w = np.concatenate([side, np.array([1.0], dtype=F32), side[::-1]])
    A = np.zeros((T, T), dtype=F32)
    for j in range(WIN):
        off = j - n
        A += np.diag(np.full(T - abs(off), w[j], dtype=F32), k=off)
    A /= np.sqrt(F32(DH))
    A1 = A.sum(axis=1).astype(F32)
    ATb = np.zeros((NT, 3, 128, 128), dtype=F32)
    for jt in range(NT):
        for s in range(3):
            rt = jt - 1 + s
            if 0 <= rt < NT:
                # rhs[q, q'] for out-tile jt, K-tile rt: A[q, q'] (A symmetric)
                ATb[jt, s] = A[rt * 128:(rt + 1) * 128, jt * 128:(jt + 1) * 128]
    return ATb, A1


def _emit(tc, ins, outs):
    import concourse.bass as bass
    import concourse.mybir as mybir
    from concourse.masks import make_identity

    nc = tc.nc
    dt = mybir.dt.float32
    x_d, wq_d, wk_d, wv_d, wo_d, bq_d, bk_d, bv_d, bo_d, atb_d, a1_d = ins
    y_d = outs

    from contextlib import ExitStack
    ctx = ExitStack()
    cn = ctx.enter_context(tc.tile_pool(name="cn", bufs=1))

    # ---- constants / inputs to SBUF ----
    x_sb = cn.tile([128, NT, DIM], dt)          # x[j*128+p, c]
    nc.sync.dma_start(out=x_sb, in_=x_d.rearrange("(j p) c -> p j c", p=128))
    atb_sb = cn.tile([128, NT, 3, 128], dt)
    nc.sync.dma_start(out=atb_sb, in_=atb_d.rearrange("j s p c -> p j s c"))
    wq_sb = cn.tile([128, NC, DIM], dt)
    nc.sync.dma_start(out=wq_sb, in_=wq_d.rearrange("(a p) i -> p a i", p=128))
    wk_sb = cn.tile([128, NC, DIM], dt)
    nc.sync.dma_start(out=wk_sb, in_=wk_d.rearrange("(a p) i -> p a i", p=128))
    wv_sb = cn.tile([128, NC, DIM], dt)
    nc.sync.dma_start(out=wv_sb, in_=wv_d.rearrange("(a p) i -> p a i", p=128))
    wo_sb = cn.tile([128, NC, DIM], dt)
    nc.sync.dma_start(out=wo_sb, in_=wo_d.rearrange("(a p) i -> p a i", p=128))
    a1_sb = cn.tile([1, T], dt)
    nc.sync.dma_start(out=a1_sb, in_=a1_d)
    bq_sb = cn.tile([1, DIM], dt)
    nc.sync.dma_start(out=bq_sb, in_=bq_d)
    bv_sb = cn.tile([1, DIM], dt)
    nc.sync.dma_start(out=bv_sb, in_=bv_d)
    bo_sb = cn.tile([1, DIM], dt)
    nc.sync.dma_start(out=bo_sb, in_=bo_d)
    bk_sb = cn.tile([128, NC], dt)              # bk[a*128+p] -> [p, a]
    nc.sync.dma_start(out=bk_sb, in_=bk_d.rearrange("(a p) -> p a", p=128))

    ident = cn.tile([128, 128], dt)
    make_identity(nc, ident[:])
    ones_sb = cn.tile([1, 128], dt)
    nc.vector.memset(ones_sb, 1.0)

    # ---- big persistent intermediates ----
    xT_sb = cn.tile([128, NC, T], dt)    # x.T   [c, t]
    xtT_sb = cn.tile([128, NC, T], dt)   # x~.T  [c, q']
    qtT_sb = cn.tile([128, NC, T], dt)   # Q~.T  [i, q']
    kT_sb = cn.tile([128, NC, T], dt)    # K.T   [i, k]
    VA = 65
    v_sb = cn.tile([128, NT, H * VA], dt)  # V augmented: per head 64 vals + ones col
    ov = v_sb.rearrange("p j (h u) -> p j h u", u=VA)
    nc.vector.memset(ov[:, :, :, 64:65], 1.0)
    oT_sb = cn.tile([128, NC, T], dt)    # normalized attention out, T-layout [i, t]

    with tc.tile_pool(name="ps_a", bufs=4, space="PSUM") as ps_a:
        # ---- phase 1: xT = x.T via PE identity matmuls ----
        for ct in range(NC):
            for jg in range(2):  # groups of 4 token tiles -> one [128, 512] psum
                p = ps_a.tile([128, 512], dt)
                for q in range(4):
                    jt = jg * 4 + q
                    nc.tensor.matmul(p[:, q * 128:(q + 1) * 128],
                                     x_sb[:, jt, ct * 128:(ct + 1) * 128],
                                     ident[:], start=True, stop=True)
                nc.vector.tensor_copy(xT_sb[:, ct, jg * 512:(jg + 1) * 512], p[:])

        # ---- phase 2: x~T[c, q'] = sum_q x[q, c] * (A/8)[q, q'] (banded) ----
        for ct in range(NC):
            for jg in range(2):
                p = ps_a.tile([128, 512], dt)
                for q in range(4):
                    jt = jg * 4 + q
                    svalid = [s for s in range(3) if 0 <= jt - 1 + s < NT]
                    for si, s in enumerate(svalid):
                        nc.tensor.matmul(
                            p[:, q * 128:(q + 1) * 128],
                            x_sb[:, jt - 1 + s, ct * 128:(ct + 1) * 128],
                            atb_sb[:, jt, s, :],
                            start=(si == 0), stop=(si == len(svalid) - 1))
                nc.vector.tensor_copy(xtT_sb[:, ct, jg * 512:(jg + 1) * 512], p[:])

        # ---- phase 3: Q~T[i, q'] = Wq.T @ x~T + bq x A1 ----
        for it in range(NC):
            for qc in range(2):
                p = ps_a.tile([128, 512], dt)
                for ct in range(NC):
                    nc.tensor.matmul(p[:], wq_sb[:, ct, it * 128:(it + 1) * 128],
                                     xtT_sb[:, ct, qc * 512:(qc + 1) * 512],
                                     start=(ct == 0), stop=False)
                nc.tensor.matmul(p[:], bq_sb[:, it * 128:(it + 1) * 128],
                                 a1_sb[:, qc * 512:(qc + 1) * 512],
                                 start=False, stop=True)
                nc.vector.tensor_copy(qtT_sb[:, it, qc * 512:(qc + 1) * 512], p[:])

        # ---- phase 4: KT[i, k] = Wk.T @ xT  (+bk per-partition on copy-out) ----
        for it in range(NC):
            for qc in range(2):
                p = ps_a.tile([128, 512], dt)
                for ct in range(NC):
                    nc.tensor.matmul(p[:], wk_sb[:, ct, it * 128:(it + 1) * 128],
                                     xT_sb[:, ct, qc * 512:(qc + 1) * 512],
                                     start=(ct == 0), stop=(ct == NC - 1))
                nc.vector.tensor_scalar(
                    out=kT_sb[:, it, qc * 512:(qc + 1) * 512], in0=p[:],
                    scalar1=bk_sb[:, it:it + 1], scalar2=None,
                    op0=mybir.AluOpType.add)

        # ---- phase 5: V[t, i] = xT.T @ Wv + 1 x bv ----
        for jt in range(NT):
            p = ps_a.tile([128, 512], dt)
            for ct in range(NC):
                nc.tensor.matmul(p[:], xT_sb[:, ct, jt * 128:(jt + 1) * 128],
                                 wv_sb[:, ct, :], start=(ct == 0), stop=False)
            nc.tensor.matmul(p[:], ones_sb[:], bv_sb[:], start=False, stop=True)
            nc.vector.tensor_copy(
                ov[:, jt, :, 0:64],
                p[:].rearrange("p (h u) -> p h u", u=64))

    # ---- phase 6: per-head attention, heads in pairs (row-packed on PE) ----
    with tc.tile_pool(name="ps_st", bufs=2, space="PSUM") as ps_st, \
         tc.tile_pool(name="ps_av", bufs=2, space="PSUM") as ps_av, \
         tc.tile_pool(name="pexp", bufs=3) as pexp, \
         tc.tile_pool(name="dn", bufs=2) as dn:
        for g in range(H // 2):
            avs = {}
            for h in (2 * g, 2 * g + 1):
                av_t = ps_av.tile([128, T], mybir.dt.float32, tag="av", name=f"av{h}")
                avs[h] = av_t
            for kt in range(NT):
                for h in (2 * g, 2 * g + 1):
                    hb = (h % 2) * 64
                    it = h // 2
                    st = ps_st.tile([128, T], mybir.dt.float32, tag="st")
                    for qc in range(2):
                        nc.tensor.matmul(
                            st[:, qc * 512:(qc + 1) * 512],
                            kT_sb[hb:hb + 64, it, kt * 128:(kt + 1) * 128],
                            qtT_sb[hb:hb + 64, it, qc * 512:(qc + 1) * 512],
                            start=True, stop=True)
                    pe = pexp.tile([128, T], mybir.dt.float32)
                    nc.scalar.activation(out=pe[:], in_=st[:],
                                         func=mybir.ActivationFunctionType.Exp)
                    for qc in range(2):
                        nc.tensor.matmul(
                            avs[h][0:VA, qc * 512:(qc + 1) * 512],
                            v_sb[:, kt, h * VA:(h + 1) * VA],
                            pe[:, qc * 512:(qc + 1) * 512],
                            start=(kt == 0), stop=(kt == NT - 1))
            for h in (2 * g, 2 * g + 1):
                hb = (h % 2) * 64
                it = h // 2
                den = dn.tile([1, T], mybir.dt.float32, tag="den")
                nc.vector.tensor_copy(den[:], avs[h][64:65, :])
                rcp = dn.tile([1, T], mybir.dt.float32, tag="rcp")
                nc.vector.reciprocal_approx_fast(out=rcp[:], in_=den[:])
                bc = dn.tile([64, T], mybir.dt.float32, tag="bc")
                nc.gpsimd.partition_broadcast(bc[:], rcp[:])
                nc.vector.tensor_mul(oT_sb[hb:hb + 64, it, :], avs[h][0:64, :],
                                     bc[:])

    # ---- phase 7: y = oT.T @ Wo + 1 x bo ----
    with tc.tile_pool(name="ps_y", bufs=4, space="PSUM") as ps_y, \
         tc.tile_pool(name="ysb", bufs=4) as ysb:
        for jt in range(NT):
            p = ps_y.tile([128, 512], mybir.dt.float32)
            for it in range(NC):
                nc.tensor.matmul(p[:], oT_sb[:, it, jt * 128:(jt + 1) * 128],
                                 wo_sb[:, it, :], start=(it == 0), stop=False)
            nc.tensor.matmul(p[:], ones_sb[:], bo_sb[:], start=False, stop=True)
            yt = ysb.tile([128, 512], mybir.dt.float32)
            nc.vector.tensor_copy(yt[:], p[:])
            nc.sync.dma_start(out=y_d[jt * 128:(jt + 1) * 128, :], in_=yt[:])

    ctx.close()


_CACHE = {}


def _get_program(reps=1):
    if ("nc", reps) in _CACHE:
        return _CACHE[("nc", reps)]
    import concourse.mybir as mybir
    import concourse.tile as tile
    from concourse import bacc

    nc = bacc.Bacc("TRN2", target_bir_lowering=False, debug=False)
    dt = mybir.dt.float32

    def din(name, shape):
        return nc.dram_tensor(name, shape, dt, kind="ExternalInput").ap()

    ins = (
        din("x", [T, DIM]),
        din("Wq", [DIM, DIM]), din("Wk", [DIM, DIM]), din("Wv", [DIM, DIM]),
        din("Wo", [DIM, DIM]),
        din("bq", [1, DIM]), din("bk", [DIM]), din("bv", [1, DIM]),
        din("bo", [1, DIM]),
        din("ATb", [NT, 3, 128, 128]), din("A1", [1, T]),
    )
    y = nc.dram_tensor("y", [T, DIM], dt, kind="ExternalOutput").ap()

    with tile.TileContext(nc) as tc:
        for _ in range(reps):
            _emit(tc, ins, y)
    nc.compile()
    _CACHE[("nc", reps)] = nc
    return nc


def _prep_in_maps(inputs):
    ATb, A1 = _band_consts()
    x = np.ascontiguousarray(inputs["x"], dtype=F32)
    shared = {
        "Wq": np.ascontiguousarray(inputs["Wq"], F32),
        "Wk": np.ascontiguousarray(inputs["Wk"], F32),
        "Wv": np.ascontiguousarray(inputs["Wv"], F32),
        "Wo": np.ascontiguousarray(inputs["Wo"], F32),
        "bq": np.ascontiguousarray(inputs["bq"], F32).reshape(1, DIM),
        "bk": np.ascontiguousarray(inputs["bk"], F32),
        "bv": np.ascontiguousarray(inputs["bv"], F32).reshape(1, DIM),
        "bo": np.ascontiguousarray(inputs["bo"], F32).reshape(1, DIM),
        "ATb": ATb, "A1": A1.reshape(1, T),
    }
    return [{"x": x[b], **shared} for b in range(B)]


def kernel(**inputs):
    from concourse import bass_utils

    nc = _get_program()
    in_maps = _prep_in_maps(inputs)
    res = bass_utils.run_bass_kernel_spmd(nc, in_maps, core_ids=list(range(B)))
    out = np.stack([res.results[b]["y"] for b in range(B)], axis=0)
    return out.astype(F32)


# revision 14
# speedup vs baseline: 604.7146x; 6.3893x over previous
"""Trainium2 Bass kernel for nn_MultiHeadAttention_26225070309648 (sparse_attention).

Full (unsharded) inputs in, full output out. Data-parallel over batch: each of
the 8 NeuronCores processes one batch element (B == 8).

Key algebraic transform: the reference applies a 31-tap conv along the QUERY
axis of the scores (then softmax). Conv over q commutes with the k-contraction,
so conv(scores) == (A @ Q) @ K.T where A is the banded [T, T] window matrix
(zero-padded at the edges, exactly matching conv2d zero padding). Folding the
1/sqrt(dh) scale into A:  S_final = (A/8 @ (x Wq + 1 bq)) @ K.T
                                  = (x~ Wq + A1 x bq) @ K.T
with x~ = (A/8) @ x computed once per core (banded matmul, cheap) and
A1 = (A/8) @ 1 (row sums). Scores are tiny (|S| < 5), so softmax needs no
max-subtraction; the denominator comes for free as a 65th "ones" column of V.

All matmuls run in float32r (TF32-like, 1 cycle/row for N>=256 vs 4 for fp32;
measured per-matmul rel err ~1.6e-4, end-to-end ~1e-3 of output absmax).
"""
import numpy as np

T = 1024
DIM = 512
H = 8
DH = 64
WIN = 31
FRAC = 0.3
B = 8
NT = T // 128   # 8 token tiles
NC = DIM // 128  # 4 channel tiles
F32 = np.float32


def _band_consts():
    """ATc[c, s] = (A/8)[(2c-1+s)*128 : .., c*256 : ..] block (zeros out of
    range), for N=256 output chunks of the banded x~ matmul;
    A1 = (A/8) @ ones (row sums of the zero-padded band)."""
    n = (WIN - 1) // 2
    side = np.array([(1.0 - FRAC) ** i for i in range(n, 0, -1)], dtype=F32)
    w = np.concatenate([side, np.array([1.0], dtype=F32), side[::-1]])
    A = np.zeros((T, T), dtype=F32)
    for j in range(WIN):
        off = j - n
        A += np.diag(np.full(T - abs(off), w[j], dtype=F32), k=off)
    A /= np.sqrt(F32(DH))
    A1 = A.sum(axis=1).astype(F32)
    NCH = T // 256  # 4 chunks
    ATc = np.zeros((NCH, 4, 128, 256), dtype=F32)
    for c in range(NCH):
        for s in range(4):
            rt = 2 * c - 1 + s
            if 0 <= rt < NT:
                ATc[c, s] = A[rt * 128:(rt + 1) * 128, c * 256:(c + 1) * 256]
    return ATc, A1


def _emit(tc, ins, outs):
    import concourse.mybir as mybir
    from concourse.masks import make_identity

    nc = tc.nc
    dt = mybir.dt.float32
    dtr = mybir.dt.float32r
    x_d, wq_d, wk_d, wv_d, wo_d, bq_d, bk_d, bv_d, bo_d, atc_d, a1_d = ins
    y_d = outs

    from contextlib import ExitStack
    ctx = ExitStack()
    cn = ctx.enter_context(tc.tile_pool(name="cn", bufs=1))

    # ---- constants / inputs to SBUF (fp32r via bitcast; HW rounds on use) ----
    def load(tile_ap, dram_ap):
        nc.sync.dma_start(out=tile_ap, in_=dram_ap.bitcast(dtr))

    tmp = tc.alloc_tile_pool(name="tmp", bufs=1)
    x_sb = tmp.tile([128, NT, DIM], dtr)          # x[j*128+p, c]
    load(x_sb, x_d.rearrange("(j p) c -> p j c", p=128))
    atc_sb = tmp.tile([128, 4, 4, 256], dtr)
    load(atc_sb, atc_d.rearrange("c s p q -> p c s q"))
    wq_sb = tmp.tile([128, NC, DIM], dtr)
    load(wq_sb, wq_d.rearrange("(a p) i -> p a i", p=128))
    wk_sb = tmp.tile([128, NC, DIM], dtr)
    load(wk_sb, wk_d.rearrange("(a p) i -> p a i", p=128))
    wv_sb = tmp.tile([128, NC, DIM], dtr)
    load(wv_sb, wv_d.rearrange("(a p) i -> p a i", p=128))
    wo_sb = cn.tile([128, NC, DIM], dtr)
    load(wo_sb, wo_d.rearrange("(a p) i -> p a i", p=128))
    a1_sb = cn.tile([1, T], dtr)
    load(a1_sb, a1_d)
    bq_sb = cn.tile([1, DIM], dtr)
    load(bq_sb, bq_d)
    bv_sb = cn.tile([1, DIM], dtr)
    load(bv_sb, bv_d)
    bo_sb = cn.tile([1, DIM], dtr)
    load(bo_sb, bo_d)
    bk_sb = cn.tile([128, NC], dt)              # bk[a*128+p] -> [p, a]
    nc.sync.dma_start(out=bk_sb, in_=bk_d.rearrange("(a p) -> p a", p=128))

    identf = cn.tile([128, 128], dt)
    make_identity(nc, identf[:])
    ident = cn.tile([128, 128], dtr)
    nc.vector.tensor_copy(ident[:], identf[:])
    onesf = cn.tile([1, 128], dt)
    nc.vector.memset(onesf, 1.0)
    ones_sb = cn.tile([1, 128], dtr)
    nc.vector.tensor_copy(ones_sb[:], onesf[:])
    ones_col = cn.tile([128, 1], dt)
    nc.vector.memset(ones_col, 1.0)

    # ---- big persistent intermediates (all feed matmuls -> fp32r) ----
    xT_sb = tmp.tile([128, NC, T], dtr)    # x.T   [c, t]
    xtT_sb = tmp.tile([128, NC, T], dtr)   # x~.T  [c, q']
    qtT_sb = cn.tile([128, NC, T], dtr)   # Q~.T  [i, q']
    kT_sb = cn.tile([128, NC, T], dtr)    # K.T   [i, k]
    VA = 65
    v_sb = cn.tile([128, NT, H * VA], dtr)  # V aug: per head 64 vals + ones col
    ov = v_sb.rearrange("p j (h u) -> p j h u", u=VA)
    for jt in range(NT):
        nc.vector.tensor_copy(ov[:, jt, :, 64:65],
                              ones_col[:].to_broadcast((128, H, 1)))
    oT_sb = cn.tile([128, NC, T], dtr)    # normalized attention out [i, t]

    with tc.tile_pool(name="ps_a", bufs=4, space="PSUM") as ps_a:
        # ---- phase 1: xT = x.T via PE transposes ----
        for ct in range(NC):
            for jg in range(2):  # groups of 4 token tiles -> one [128, 512] psum
                p = ps_a.tile([128, 512], dtr, tag="ps", name=f"p1_{ct}_{jg}")
                for q in range(4):
                    jt = jg * 4 + q
                    nc.tensor.transpose(p[:, q * 128:(q + 1) * 128],
                                        x_sb[:, jt, ct * 128:(ct + 1) * 128],
                                        ident[:])
                nc.vector.tensor_copy(xT_sb[:, ct, jg * 512:(jg + 1) * 512], p[:])

        # ---- phase 2: x~T[c, q'] = sum_q x[q, c] (A/8)[q, q'] (banded, N=256) ----
        for ct in range(NC):
            for cc in range(4):
                p = ps_a.tile([128, 256], dt, tag="ps", name=f"p2_{ct}_{cc}")
                svalid = [s for s in range(4) if 0 <= 2 * cc - 1 + s < NT]
                for si, s in enumerate(svalid):
                    nc.tensor.matmul(
                        p[:],
                        x_sb[:, 2 * cc - 1 + s, ct * 128:(ct + 1) * 128],
                        atc_sb[:, cc, s, :],
                        start=(si == 0), stop=(si == len(svalid) - 1))
                nc.vector.tensor_copy(
                    xtT_sb[:, ct, cc * 256:(cc + 1) * 256], p[:])

        # ---- phase 3: Q~T[i, q'] = Wq.T @ x~T + bq x A1 ----
        for it in range(NC):
            for qc in range(2):
                p = ps_a.tile([128, 512], dt, tag="ps", name=f"p3_{it}_{qc}")
                for ct in range(NC):
                    nc.tensor.matmul(p[:], wq_sb[:, ct, it * 128:(it + 1) * 128],
                                     xtT_sb[:, ct, qc * 512:(qc + 1) * 512],
                                     start=(ct == 0), stop=False)
                nc.tensor.matmul(p[:], bq_sb[:, it * 128:(it + 1) * 128],
                                 a1_sb[:, qc * 512:(qc + 1) * 512],
                                 start=False, stop=True)
                nc.vector.tensor_copy(qtT_sb[:, it, qc * 512:(qc + 1) * 512], p[:])

        # ---- phase 4: KT[i, k] = Wk.T @ xT  (+bk per-partition on copy-out) ----
        for it in range(NC):
            for qc in range(2):
                p = ps_a.tile([128, 512], dt, tag="ps", name=f"p4_{it}_{qc}")
                for ct in range(NC):
                    nc.tensor.matmul(p[:], wk_sb[:, ct, it * 128:(it + 1) * 128],
                                     xT_sb[:, ct, qc * 512:(qc + 1) * 512],
                                     start=(ct == 0), stop=(ct == NC - 1))
                nc.vector.tensor_scalar(
                    out=kT_sb[:, it, qc * 512:(qc + 1) * 512], in0=p[:],
                    scalar1=bk_sb[:, it:it + 1], scalar2=None,
                    op0=mybir.AluOpType.add)

        # ---- phase 5: V[t, i] = xT.T @ Wv + 1 x bv ----
        for jt in range(NT):
            p = ps_a.tile([128, 512], dt, tag="ps", name=f"p5_{jt}")
            for ct in range(NC):
                nc.tensor.matmul(p[:], xT_sb[:, ct, jt * 128:(jt + 1) * 128],
                                 wv_sb[:, ct, :], start=(ct == 0), stop=False)
            nc.tensor.matmul(p[:], ones_sb[:], bv_sb[:], start=False, stop=True)
            nc.vector.tensor_copy(
                ov[:, jt, :, 0:64],
                p[:].rearrange("p (h u) -> p h u", u=64))

    tmp.release()

    # ---- phase 6: per-head attention, heads in pairs (row-packed on PE) ----
    with tc.tile_pool(name="ps_st", bufs=2, space="PSUM") as ps_st, \
         tc.tile_pool(name="ps_av", bufs=2, space="PSUM") as ps_av, \
         tc.tile_pool(name="pexp", bufs=3) as pexp, \
         tc.tile_pool(name="dn", bufs=2) as dn:
        for g in range(H // 2):
            avs = {}
            for h in (2 * g, 2 * g + 1):
                av_t = ps_av.tile([128, T], mybir.dt.float32, tag="av",
                                  name=f"av{h}")
                avs[h] = av_t
            for kt in range(NT):
                for h in (2 * g, 2 * g + 1):
                    hb = (h % 2) * 64
                    it = h // 2
                    st = ps_st.tile([128, T], mybir.dt.float32, tag="st")
                    for qc in range(2):
                        nc.tensor.matmul(
                            st[:, qc * 512:(qc + 1) * 512],
                            kT_sb[hb:hb + 64, it, kt * 128:(kt + 1) * 128],
                            qtT_sb[hb:hb + 64, it, qc * 512:(qc + 1) * 512],
                            start=True, stop=True)
                    pe = pexp.tile([128, T], mybir.dt.float32r)
                    nc.scalar.activation(out=pe[:], in_=st[:],
                                         func=mybir.ActivationFunctionType.Exp)
                    for qc in range(2):
                        nc.tensor.matmul(
                            avs[h][0:VA, qc * 512:(qc + 1) * 512],
                            v_sb[:, kt, h * VA:(h + 1) * VA],
                            pe[:, qc * 512:(qc + 1) * 512],
                            start=(kt == 0), stop=(kt == NT - 1))
            for h in (2 * g, 2 * g + 1):
                hb = (h % 2) * 64
                it = h // 2
                den = dn.tile([1, T], mybir.dt.float32, tag="den")
                nc.vector.tensor_copy(den[:], avs[h][64:65, :])
                rcp = dn.tile([1, T], mybir.dt.float32, tag="rcp")
                nc.vector.reciprocal_approx_fast(out=rcp[:], in_=den[:])
                bc = dn.tile([64, T], mybir.dt.float32, tag="bc")
                nc.gpsimd.partition_broadcast(bc[:], rcp[:])
                nc.vector.tensor_mul(oT_sb[hb:hb + 64, it, :], avs[h][0:64, :],
                                     bc[:])

    # ---- phase 7: y = oT.T @ Wo + 1 x bo ----
    with tc.tile_pool(name="ps_y", bufs=4, space="PSUM") as ps_y, \
         tc.tile_pool(name="ysb", bufs=4) as ysb:
        for jt in range(NT):
            p = ps_y.tile([128, 512], mybir.dt.float32)
            for it in range(NC):
                nc.tensor.matmul(p[:], oT_sb[:, it, jt * 128:(jt + 1) * 128],
                                 wo_sb[:, it, :], start=(it == 0), stop=False)
            nc.tensor.matmul(p[:], ones_sb[:], bo_sb[:], start=False, stop=True)
            yt = ysb.tile([128, 512], mybir.dt.float32)
            nc.vector.tensor_copy(yt[:], p[:])
            nc.sync.dma_start(out=y_d[jt * 128:(jt + 1) * 128, :], in_=yt[:])

    ctx.close()


_CACHE = {}


def _get_program(reps=1):
    if ("nc", reps) in _CACHE:
        return _CACHE[("nc", reps)]
    import concourse.mybir as mybir
    import concourse.tile as tile
    from concourse import bacc

    nc = bacc.Bacc("TRN2", target_bir_lowering=False, debug=False)
    dt = mybir.dt.float32

    def din(name, shape):
        return nc.dram_tensor(name, shape, dt, kind="ExternalInput").ap()

    ins = (
        din("x", [T, DIM]),
        din("Wq", [DIM, DIM]), din("Wk", [DIM, DIM]), din("Wv", [DIM, DIM]),
        din("Wo", [DIM, DIM]),
        din("bq", [1, DIM]), din("bk", [DIM]), din("bv", [1, DIM]),
        din("bo", [1, DIM]),
        din("ATc", [T // 256, 4, 128, 256]), din("A1", [1, T]),
    )
    y = nc.dram_tensor("y", [T, DIM], dt, kind="ExternalOutput").ap()

    with tile.TileContext(nc) as tc:
        for _ in range(reps):
            _emit(tc, ins, y)
    nc.compile()
    _CACHE[("nc", reps)] = nc
    return nc


def _prep_in_maps(inputs):
    ATc, A1 = _band_consts()
    x = np.ascontiguousarray(inputs["x"], dtype=F32)
    shared = {
        "Wq": np.ascontiguousarray(inputs["Wq"], F32),
        "Wk": np.ascontiguousarray(inputs["Wk"], F32),
        "Wv": np.ascontiguousarray(inputs["Wv"], F32),
        "Wo": np.ascontiguousarray(inputs["Wo"], F32),
        "bq": np.ascontiguousarray(inputs["bq"], F32).reshape(1, DIM),
        "bk": np.ascontiguousarray(inputs["bk"], F32),
        "bv": np.ascontiguousarray(inputs["bv"], F32).reshape(1, DIM),
        "bo": np.ascontiguousarray(inputs["bo"], F32).reshape(1, DIM),
        "ATc": ATc, "A1": A1.reshape(1, T),
    }
    return [{"x": x[b], **shared} for b in range(B)]


def kernel(**inputs):
    from concourse import bass_utils

    nc = _get_program()
    in_maps = _prep_in_maps(inputs)
    res = bass_utils.run_bass_kernel_spmd(nc, in_maps, core_ids=list(range(B)))
    out = np.stack([res.results[b]["y"] for b in range(B)], axis=0)
    return out.astype(F32)
